# revision 7
# baseline (speedup 1.0000x reference)
"""NetVLAD forward kernel for 8 TRN2 NeuronCores (Bass/Tile).

Reference (per batch b of 32):
  s = x @ Wk + b         (1024, 64) logits;  softmax over k -> a
  v[d,k] = sum_n a[n,k] x[n,d] + (sum_n a[n,k]) * C[d,k]
  v /= ||v||_2 over d (per k);  out = flatten(v) / ||flatten(v)||_2

Sharding: data-parallel over batch B=32 across 8 cores (4 batches/core).
Wk, b, C replicated; no collectives; host concatenates outputs.

Design (v4):
  - Host ships x twice in SBUF-exact layouts: xn bf16 (pixels on
    partitions, mm2 moving) + xt8 fp8e3m4 (d on partitions, mm1 moving).
    No on-chip x transposes or casts; 6MB/core HBM traffic. fp8 on the
    logits path costs ~2x bf16 rel-err (~4e-3 vs gate 2e-2); Wk stays
    bf16 (0.02-scale weights are subnormal in fp8).
  - One DMA per (tensor, batch) — 8 x-triggers split over the SP and ACT
    HWDGE queues (trigger issue ~1us each serializes a queue; 16 triggers
    on one queue stretched the input phase in v3).
  - mm1 per n-half into s^T[64k,512n] (halves pack one PSUM bank); exp
    per half (bias=b2); 4 e-transposes/half back to a-natural; 1/Z folded
    into a with a single broadcast tensor_mul per half (DVE chain must
    stay shorter than mm1 of the next half or the PE stalls).
  - Batch emission is two-phase (all mm1/eT, then all mm2) so the PE
    runs h1's mm1 while the DVE does h0's softmax.
  - mm2: a chunks stationary, xn moving 512 wide; batch pairs pack
    v/asum PSUM rows. asum via ones-column matmuls.
  - Tail: S_k per pair by ACT Square+accum_out right after the odd
    batch; pair0's sqrt/scale/transpose/store are emitted after b3 so
    the single Exp->Sqrt table load (1.28us) and all of pair0's output
    chain hide under b3's compute; only pair1's short chain is serial.
    Global norm folded as 1/(8*sqrt(S+eps)); output stored bf16 ([d,k]
    via PE transposes), host upcasts to f32.
  - 28 warmup matmuls on the identity while DMAs land release the PE HAM
    clock gate (1.2 -> 2.4 GHz) before real work arrives.
"""

import sys

sys.path.insert(0, "/opt/trn_rl_repo")

from contextlib import ExitStack

import numpy as np

import concourse.bacc as bacc
import concourse.tile as tile
from concourse import mybir
from concourse.bass_utils import run_bass_kernel_spmd

F32 = mybir.dt.float32
BF16 = mybir.dt.bfloat16
FP8 = mybir.dt.float8e3
AX = mybir.AxisListType
ACTF = mybir.ActivationFunctionType

B_PER_CORE = 4  # 32 batches / 8 cores
N = 1024  # H*W pixels per batch
D = 512
K = 64
EPS = 1e-12
N_CORES = 8
N_WARM = 28


def build_kernel():
    nc = bacc.Bacc()
    xt8_d = nc.declare_dram_parameter("xt8", [128, 4, 2, 4, 512], FP8, isOutput=False)
    xn_d = nc.declare_dram_parameter("xn", [128, 4, 2, 4, 512], BF16, isOutput=False)
    wkb_d = nc.declare_dram_parameter("wkb", [128, 4, K], BF16, isOutput=False)
    b2_d = nc.declare_dram_parameter("b2", [128, 1], F32, isOutput=False)
    ct2_d = nc.declare_dram_parameter("ct2", [128, D], F32, isOutput=False)
    idbf_d = nc.declare_dram_parameter("idbf", [128, 128], BF16, isOutput=False)
    out_d = nc.declare_dram_parameter("out", [4, 128, 4, K], BF16, isOutput=True)

    with tile.TileContext(nc) as tc, ExitStack() as ctx:
        const = ctx.enter_context(tc.tile_pool(name="const", bufs=1))
        xin = ctx.enter_context(tc.tile_pool(name="xin", bufs=1))
        sb = ctx.enter_context(tc.tile_pool(name="sb", bufs=3))
        nrm = ctx.enter_context(tc.tile_pool(name="nrm", bufs=2))
        ps_s = ctx.enter_context(tc.tile_pool(name="ps_s", bufs=2, space="PSUM"))
        ps_e = ctx.enter_context(tc.tile_pool(name="ps_e", bufs=1, space="PSUM"))
        ps_v = ctx.enter_context(tc.tile_pool(name="ps_v", bufs=2, space="PSUM"))
        ps_as = ctx.enter_context(tc.tile_pool(name="ps_as", bufs=1, space="PSUM"))
        ps_o = ctx.enter_context(tc.tile_pool(name="ps_o", bufs=1, space="PSUM"))
        ps_w = ctx.enter_context(tc.tile_pool(name="ps_w", bufs=1, space="PSUM"))

        # ---- constants + x loads: xt on the ACT queue, xn on the SP queue ----
        idbf = const.tile([128, 128], BF16)
        nc.scalar.dma_start(out=idbf[:], in_=idbf_d[:])
        wkb = const.tile([128, 4, K], BF16)
        nc.scalar.dma_start(out=wkb[:], in_=wkb_d[:])
        b2 = const.tile([128, 1], F32)
        nc.scalar.dma_start(out=b2[:], in_=b2_d[:])
        ct2 = const.tile([128, D], F32)
        ones = const.tile([128, 1], BF16)
        nc.vector.memset(ones[:], 1.0)
        eps64 = const.tile([128, 1], F32)
        nc.vector.memset(eps64[:], float(64 * EPS))
        S_all = const.tile([128, 2], F32)

        xt_all = xin.tile([128, 4, 2, 4, 512], FP8)
        xn_all = xin.tile([128, 4, 2, 4, 512], BF16)
        for b in range(B_PER_CORE):
            nc.scalar.dma_start(out=xt_all[:, b], in_=xt8_d[:, b])
            nc.sync.dma_start(out=xn_all[:, b], in_=xn_d[:, b])
            if b == 1:
                nc.scalar.dma_start(out=ct2[:], in_=ct2_d[:])

        # ---- PE warmup: release the HAM clock gate while DMAs land ----
        warm = ps_w.tile([128, 128], F32)
        for _ in range(N_WARM):
            nc.tensor.matmul(warm[:], idbf[:], idbf[:], start=True, stop=True)

        # ---- per-batch pipeline ----
        v2 = {}
        vvs = {}
        for b in range(B_PER_CORE):
            p2, h2 = b // 2, b % 2
            s_ps = ps_s.tile([128, 512], F32, tag="s")
            eT = sb.tile([128, 512], BF16, tag="eT")
            e_ps = ps_e.tile([128, 8, K], BF16, tag="e")
            z = sb.tile([128, 8], F32, tag="z")
            invz = sb.tile([128, 8], F32, tag="invz")
            a_sb = sb.tile([128, 8, K], BF16, tag="a")
            if h2 == 0:
                v_ps = ps_v.tile([128, 512], F32, tag="v")
                as_ps = ps_as.tile([128, 1], F32, tag="as")
                v2[p2] = (v_ps, as_ps)
            v_ps, as_ps = v2[p2]
            # phase 1 — mm1 + softmax per n-half (h1's mm1/eT on the PE
            # overlap h0's exp/Z/a chain on ACT+DVE)
            for h in range(2):
                for j in range(4):
                    nc.tensor.matmul(
                        s_ps[64 * h : 64 * (h + 1), :],
                        wkb[:, j, :],
                        xt_all[:, b, h, j, :],
                        start=(j == 0),
                        stop=(j == 3),
                        skip_group_check=True,
                    )
                nc.scalar.activation(
                    eT[64 * h : 64 * (h + 1), :],
                    s_ps[64 * h : 64 * (h + 1), :],
                    ACTF.Exp,
                    bias=b2[64 * h : 64 * (h + 1), :],
                )
                for c in range(4):
                    nc.tensor.transpose(
                        e_ps[:, 4 * h + c, :],
                        eT[64 * h : 64 * (h + 1), c * 128 : (c + 1) * 128],
                        idbf[64 * h : 64 * (h + 1), 64 * h : 64 * (h + 1)],
                    )
                hs = slice(4 * h, 4 * (h + 1))
                nc.vector.reduce_sum(z[:, hs], e_ps[:, hs, :], axis=AX.X)
                nc.vector.reciprocal(invz[:, hs], z[:, hs])
                nc.vector.tensor_mul(
                    a_sb[:, hs, :],
                    e_ps[:, hs, :],
                    invz[:, hs].broadcast_to([128, 4, K]),
                )
            # phase 2 — mm2 + asum into the pair-packed PSUM rows
            for h in range(2):
                for c in range(4):
                    nc.tensor.matmul(
                        v_ps[64 * h2 : 64 * (h2 + 1), :],
                        a_sb[:, 4 * h + c, :],
                        xn_all[:, b, h, c, :],
                        start=(h == 0 and c == 0),
                        stop=(h == 1 and c == 3),
                        skip_group_check=True,
                    )
                    nc.tensor.matmul(
                        as_ps[64 * h2 : 64 * (h2 + 1), :],
                        a_sb[:, 4 * h + c, :],
                        ones[:],
                        start=(h == 0 and c == 0),
                        stop=(h == 1 and c == 3),
                        skip_group_check=True,
                    )

            if h2 == 1:
                # pair tail part 1: v = v_raw + asum*C^T; S = sum_d v^2
                asum2 = nrm.tile([128, 1], F32, tag=f"as{p2}")
                nc.vector.tensor_copy(asum2[:], as_ps[:])
                vc = nrm.tile([128, D], F32, tag="vc")
                nc.vector.tensor_scalar_mul(vc[:], ct2[:], asum2[:, 0:1])
                vv = nrm.tile([128, D], F32, tag=f"vv{p2}")
                nc.vector.tensor_add(vv[:], vc[:], v_ps[:])
                vvs[p2] = vv
                vsq = nrm.tile([128, D], F32, tag="vsq")
                nc.scalar.activation(
                    vsq[:], vv[:], ACTF.Square, accum_out=S_all[:, p2 : p2 + 1]
                )

        # ---- norm tails: pair0's chain hides under b3; pair1 is serial ----
        def finish_pair(p2):
            q8 = nrm.tile([128, 1], F32, tag="q8")
            nc.scalar.activation(
                q8[:], S_all[:, p2 : p2 + 1], ACTF.Sqrt, bias=eps64[:], scale=64.0
            )
            sc = nrm.tile([128, 1], F32, tag="sc")
            nc.vector.reciprocal(sc[:], q8[:])
            vfb = nrm.tile([128, D], BF16, tag="vfb")
            nc.vector.tensor_scalar_mul(vfb[:], vvs[p2][:], sc[:, 0:1])
            o_sb = nrm.tile([128, 2, 4, K], BF16, tag="osb")
            for hh in range(2):
                o_ps = ps_o.tile([128, 4, K], BF16, tag="o")
                for jj in range(4):
                    nc.tensor.transpose(
                        o_ps[:, jj, :],
                        vfb[64 * hh : 64 * (hh + 1), jj * 128 : (jj + 1) * 128],
                        idbf[64 * hh : 64 * (hh + 1), 64 * hh : 64 * (hh + 1)],
                    )
                nc.scalar.copy(o_sb[:, hh], o_ps[:])
            nc.sync.dma_start(
                out=out_d[2 * p2 : 2 * p2 + 2].rearrange("b p j k -> p b j k"),
                in_=o_sb[:],
            )

        finish_pair(0)
        finish_pair(1)

    nc.compile()
    return nc


_CACHED_NC = None


def _get_nc():
    global _CACHED_NC
    if _CACHED_NC is None:
        _CACHED_NC = build_kernel()
    return _CACHED_NC


def build_in_maps(x, Wk, b, C):
    import ml_dtypes

    B = x.shape[0]
    x2 = np.ascontiguousarray(x, dtype=np.float32).reshape(B, N, D)
    bpc = B // N_CORES
    Wkf = np.asarray(Wk, dtype=np.float32)
    Cf = np.asarray(C, dtype=np.float32)
    bf = np.asarray(b, dtype=np.float32).reshape(K)
    consts = {
        "idbf": np.eye(128).astype(ml_dtypes.bfloat16),
        "wkb": np.ascontiguousarray(
            Wkf.reshape(4, 128, K).transpose(1, 0, 2)
        ).astype(ml_dtypes.bfloat16),
        "ct2": np.ascontiguousarray(np.concatenate([Cf.T, Cf.T], axis=0)),
        "b2": np.concatenate([bf, bf]).reshape(128, 1),
    }
    in_maps = []
    for c in range(N_CORES):
        A = x2[c * bpc : (c + 1) * bpc]  # (4, 1024, 512)
        # xn[p, b, h, c, d]: pixel n = (4h+c)*128 + p
        xn = np.ascontiguousarray(
            A.reshape(bpc, 2, 4, 128, D).transpose(3, 0, 1, 2, 4)
        ).astype(ml_dtypes.bfloat16)
        # xt8[p, b, h, j, nn]: d = j*128 + p, n = h*512 + nn
        xt8 = np.ascontiguousarray(
            A.transpose(0, 2, 1).reshape(bpc, 4, 128, 2, 512).transpose(2, 0, 3, 1, 4)
        ).astype(ml_dtypes.float8_e3m4)
        in_maps.append({"xn": xn, "xt8": xt8, **consts})
    return in_maps


def kernel(x, Wk, b, C):
    """Full-input NetVLAD forward. x (32,32,32,512) f32 -> out (32, 32768) f32."""
    in_maps = build_in_maps(x, Wk, b, C)
    nc = _get_nc()
    res = run_bass_kernel_spmd(nc, in_maps, list(range(N_CORES)))
    outs = []
    for c in range(N_CORES):
        o = np.asarray(res.results[c]["out"])  # (4, 128, 4, 64) bf16
        outs.append(
            o.transpose(0, 2, 1, 3).reshape(B_PER_CORE, D * K).astype(np.float32)
        )
    return np.concatenate(outs, axis=0)
